# revision 13
# baseline (speedup 1.0000x reference)
"""Trainium2 Bass kernel for nn_Attention_39608188404100 (v4).

Windowed-attention block (ViT-style, N=197 tokens) with SSF affines, relative
position bias, DCF head mixing, and output projection.

Strategy: pure data-parallel over batch across 8 NeuronCores (B=64 -> 8/core).
All weights replicated; no collectives. bf16 on the PE, fp32 PSUM.

Per core (BL=8 batches): each batch's 197 tokens are padded to 200 positions
and PERMUTED on host: position p = c*100 + ml*10 + g holds token
m = c*100 + g*10 + ml (c = chunk, 2x100). The 3 dummy positions get zero
x-columns and exp(-40) relative-bias multipliers, so they vanish in softmax;
dummy query columns are dropped on host after download.

v4 structure (vs v3: no DRAM scratch round-trips, batch-major pipeline):
  - Stage 1: all 12 q/k channel-tile GEMMs back to back. Uploads are split
    (wqk per-mt, xT per-token-chunk) and ordered so the first matmul starts
    after ~1MB of upload.
  - Batch loop (software-pipelined): per batch, scores -> exp -> *relb ->
    denominators (ones-matmul) -> normalize produce et [100,4800] in a
    double-buffered SBUF arena; a ONE-HOP SBUF->SBUF DMA shuffle (10 j-split
    DMAs) regroups to mxin [(wgi,h),(j,cn)]; block-diag mix matmul; a second
    one-hop shuffle (12 k-split DMAs) back to key-partition layout a2 (which
    ALIASES the et region of the same arena); AV; per-128-token-chunk output
    projection merged ACROSS batches (13 chunks over 1600 tokens, full
    128-partition stationary).
  - v-projection for batch b+2 and the mix of batch b+1 are emitted between
    the shuffle DMAs of neighboring batches to keep the PE fed during DMA
    flight.

Env:
  BASS_KERNEL_PROFILE=1  capture neuron-profile (exec_time_ns) on the run.
"""
import os
import sys

sys.path.insert(0, "/opt/trn_rl_repo")

import numpy as np
import ml_dtypes

import concourse.bass as bass
import concourse.tile as tile
from concourse import bacc, mybir

BF16 = mybir.dt.bfloat16
F32 = mybir.dt.float32
AF = mybir.ActivationFunctionType
ALU = mybir.AluOpType

B, N, C, H, DH = 64, 197, 768, 12, 64
NCORES = 8
BL = B // NCORES          # 8 batches per core
P2 = 200                  # padded positions per batch
T2 = BL * P2              # 1600 positions per core
SCALE = DH ** -0.5
KT = 6                    # contraction tiles of 128 over C=768
QKM = 12                  # 128-wide M tiles over 1536 q/k channels
TOKC = [(0, 500), (500, 500), (1000, 500), (1500, 100)]  # 100-aligned chunks
DUMMY_BIAS = -40.0
ECOLS = 2 * H * P2        # 4800: et/a2 region cols
MCOLS = 10 * 2 * P2       # 4000: mxin / mxo region cols
ACOLS = ECOLS + 2 * MCOLS  # arena cols (et|a2, mxin, mxo)

_COMPILED = {}


def _build_graph():
    # detect_race_conditions=False: the sim race-detector's shadow model
    # linearizes multi-dim DMA APs as byte offsets and reports false overlaps
    # between distinct tiles; value semantics were validated in CoreSim and
    # against hardware.
    nc = bacc.Bacc(
        "TRN2", target_bir_lowering=False, debug=False,
        detect_race_conditions=False,
    )

    xT_d = nc.dram_tensor("xT", [128, KT * T2], BF16, kind="ExternalInput")
    wqk_d = nc.dram_tensor("wqk", [128, QKM * 768], BF16, kind="ExternalInput")
    wv_d = nc.dram_tensor("wv", [128, KT * 768], BF16, kind="ExternalInput")
    wp_d = nc.dram_tensor("wp", [128, KT * 768], BF16, kind="ExternalInput")
    relb_d = nc.dram_tensor("relb", [100, ECOLS], BF16, kind="ExternalInput")
    mix_d = nc.dram_tensor("mixblk", [120, 120], BF16, kind="ExternalInput")
    bqk_d = nc.dram_tensor("bqk", [128, QKM], F32, kind="ExternalInput")
    bv_d = nc.dram_tensor("bv", [128, 768], BF16, kind="ExternalInput")
    bp_d = nc.dram_tensor("bp", [128, 768], BF16, kind="ExternalInput")
    out_d = nc.dram_tensor("out", [T2, 768], BF16, kind="ExternalOutput")

    with tile.TileContext(nc) as tc:
        with (
            tc.tile_pool(name="const", bufs=1) as cpool,
            tc.tile_pool(name="dch", bufs=2) as dpool,
            tc.tile_pool(name="osb", bufs=2) as opool,
            tc.tile_pool(name="psA", bufs=2, space=bass.MemorySpace.PSUM) as psA,
            tc.tile_pool(name="psSC", bufs=2, space=bass.MemorySpace.PSUM) as psSC,
            tc.tile_pool(name="psMX", bufs=2, space=bass.MemorySpace.PSUM) as psMX,
            tc.tile_pool(name="psDV", bufs=2, space=bass.MemorySpace.PSUM) as psDV,
        ):
            # ---- persistent tiles ----
            xT = cpool.tile([128, KT * T2], BF16)
            qk_t = [cpool.tile([128, T2], BF16, name=f"qk{mt}")
                    for mt in range(QKM)]
            relb = cpool.tile([100, ECOLS], BF16)
            wv = cpool.tile([128, KT * 768], BF16)
            wp = cpool.tile([128, KT * 768], BF16)
            mixblk = cpool.tile([120, 120], BF16)
            bqk = cpool.tile([128, QKM], F32)
            bv = cpool.tile([128, 768], BF16)
            bp = cpool.tile([128, 768], BF16)
            ones_den = cpool.tile([128, 128], BF16)
            aoTall = cpool.tile([128, KT * T2], BF16)   # out^T, token-major
            # double-buffered shuffle arenas + v ring
            arena = [cpool.tile([128, ACOLS], BF16, name=f"arena{s}")
                     for s in range(2)]
            vring = [cpool.tile([100, 2 * 768], BF16, name=f"v{s}")
                     for s in range(3)]

            nc.vector.memset(ones_den[:], 1.0)
            nc.vector.memset(arena[0][:], 0.0)
            nc.vector.memset(arena[1][:], 0.0)

            def et_of(b):
                return arena[b % 2][0:100, 0:ECOLS]

            def mxin_of(b):
                return arena[b % 2][0:120, ECOLS:ECOLS + MCOLS]

            def mxo_of(b):
                return arena[b % 2][0:120, ECOLS + MCOLS:ACOLS]

            TOKBASE = [0, 3000, 6000, 9000]  # col base per token chunk (x6)

            def xt_slice(t0, kt, w):
                """xT AP for tokens [t0, t0+w) at contraction tile kt."""
                ci = min(t0 // 500, 3)
                base, off, nsz_c = TOKBASE[ci], t0 - TOKC[ci][0], TOKC[ci][1]
                col = base + kt * nsz_c + off
                return xT[:, col: col + w]

            # ---- stage 1: qkv q/k GEMMs, uploads overlapped ----
            with tc.tile_pool(name="wqk", bufs=1) as wpool:
                wqk = wpool.tile([128, QKM * 768], BF16)
                # critical-path uploads first
                nc.sync.dma_start(wqk[:, 0:768], wqk_d[:, 0:768])
                nc.sync.dma_start(xT[:, 0:3000], xT_d[:, 0:3000])
                nc.sync.dma_start(bqk[:], bqk_d[:])
                for mt in range(1, QKM):
                    nc.sync.dma_start(wqk[:, mt * 768:(mt + 1) * 768],
                                      wqk_d[:, mt * 768:(mt + 1) * 768])
                for ci in range(1, 4):
                    c0 = TOKBASE[ci]
                    c1 = TOKBASE[ci + 1] if ci < 3 else KT * T2
                    nc.sync.dma_start(xT[:, c0:c1], xT_d[:, c0:c1])
                nc.sync.dma_start(relb[:], relb_d[:])
                nc.sync.dma_start(wv[:], wv_d[:])
                nc.sync.dma_start(bv[:], bv_d[:])
                nc.sync.dma_start(mixblk[:], mix_d[:])
                nc.sync.dma_start(wp[:], wp_d[:])
                nc.sync.dma_start(bp[:], bp_d[:])

                for mt in range(QKM):
                    for ci, (n0, nsz) in enumerate(TOKC):
                        ps = psA.tile([128, 512], F32, tag="a")
                        for kt in range(KT):
                            nc.tensor.matmul(
                                ps[:, 0:nsz],
                                wqk[:, mt * 768 + kt * 128: mt * 768 + (kt + 1) * 128],
                                xT[:, TOKBASE[ci] + kt * nsz: TOKBASE[ci] + kt * nsz + nsz],
                                start=(kt == 0),
                                stop=(kt == KT - 1),
                            )
                        nc.scalar.activation(
                            qk_t[mt][:, n0:n0 + nsz],
                            ps[:, 0:nsz],
                            AF.Identity,
                            bias=bqk[:, mt:mt + 1],
                            scale=1.0,
                        )

            # ---- batch loop pieces ----
            def vproj(b):
                vt = vring[b % 3]
                for c in range(2):
                    t0 = b * P2 + c * 100
                    for n0, nsz in ((0, 512), (512, 256)):
                        ps = psA.tile([128, 512], F32, tag="a")
                        for kt in range(KT):
                            nc.tensor.matmul(
                                ps[0:100, 0:nsz],
                                xt_slice(t0, kt, 100),
                                wv[:, kt * 768 + n0: kt * 768 + n0 + nsz],
                                start=(kt == 0),
                                stop=(kt == KT - 1),
                            )
                        nc.vector.tensor_tensor(
                            vt[0:100, c * 768 + n0: c * 768 + n0 + nsz],
                            ps[0:100, 0:nsz],
                            bv[0:100, n0: n0 + nsz],
                            ALU.add,
                        )

            def sdn(b):
                """scores -> exp -> *relb -> den -> normalize for batch b."""
                et = et_of(b)
                etv = et.rearrange("p (h c n) -> p h c n", h=H, c=2, n=P2)

                def sc(tq):
                    for hh in range(4):
                        h = 4 * tq + hh
                        prow = (h % 2) * 64
                        qt = qk_t[h // 2]
                        kt_ = qk_t[6 + h // 2]
                        ps1 = psSC.tile([128, 512], F32, tag="sc")
                        nc.tensor.matmul(
                            ps1[0:100, 0:P2],
                            kt_[prow:prow + 64, b * P2: b * P2 + 100],
                            qt[prow:prow + 64, b * P2: b * P2 + P2],
                            start=True, stop=True,
                        )
                        nc.tensor.matmul(
                            ps1[0:100, P2:2 * P2],
                            kt_[prow:prow + 64, b * P2 + 100: b * P2 + 200],
                            qt[prow:prow + 64, b * P2: b * P2 + P2],
                            start=True, stop=True,
                        )
                        nc.scalar.activation(
                            et[:, h * 2 * P2:(h + 1) * 2 * P2],
                            ps1[0:100, 0:2 * P2], AF.Exp,
                        )
                    nc.vector.tensor_tensor(
                        et[:, tq * 4 * 2 * P2:(tq + 1) * 4 * 2 * P2],
                        et[:, tq * 4 * 2 * P2:(tq + 1) * 4 * 2 * P2],
                        relb[:, tq * 4 * 2 * P2:(tq + 1) * 4 * 2 * P2],
                        ALU.mult,
                    )

                def den(tq):
                    dch = dpool.tile([100, 800], F32, tag="dch")
                    for dd in range(2):
                        psd = psDV.tile([128, 512], F32, tag="dv")
                        nc.tensor.matmul(
                            psd[0:100, 0:400],
                            ones_den[0:100, 0:100],
                            etv[:, 4 * tq + 2 * dd: 4 * tq + 2 * dd + 2, 0, :],
                            start=True, stop=False,
                        )
                        nc.tensor.matmul(
                            psd[0:100, 0:400],
                            ones_den[0:100, 0:100],
                            etv[:, 4 * tq + 2 * dd: 4 * tq + 2 * dd + 2, 1, :],
                            start=False, stop=True,
                        )
                        nc.vector.reciprocal_approx_fast(
                            dch[:, dd * 400:(dd + 1) * 400],
                            psd[0:100, 0:400],
                        )
                    dv4 = dch[:].rearrange("p (h n) -> p h n", h=4)
                    for c in range(2):
                        nc.vector.tensor_tensor(
                            etv[:, 4 * tq:4 * tq + 4, c, :],
                            etv[:, 4 * tq:4 * tq + 4, c, :],
                            dv4, ALU.mult,
                        )

                sc(0)
                sc(1)
                den(0)
                sc(2)
                den(1)
                den(2)

            def hop2(b):
                """et [(j wgi), (h cn)] -> mxin [(wgi h), (j cn)], j-split.
                src = contiguous 10-partition slice; dst (wgi,h) iterated
                wgi-outer/h-inner merges to a contiguous 120-partition dim."""
                et_v = et_of(b).rearrange(
                    "(j wgi) (h cn) -> j wgi h cn", j=10, cn=2 * P2)
                mx = mxin_of(b)
                for j in range(10):
                    # dst as a plain [120, 400] rect (partition dim stride 1);
                    # element order (wgi,h),cn matches src's (wgi, h, cn).
                    nc.sync.dma_start(
                        mx[:, j * 2 * P2: (j + 1) * 2 * P2], et_v[j])

            def mix(b):
                mxin = mxin_of(b)
                mxo = mxo_of(b)
                for o in range(0, MCOLS, 500):
                    psm = psMX.tile([128, 512], F32, tag="mx")
                    nc.tensor.matmul(
                        psm[0:120, 0:500], mixblk[:],
                        mxin[:, o:o + 500],
                        start=True, stop=True,
                    )
                    nc.vector.tensor_scalar_add(
                        mxo[:, o:o + 500], psm[0:120, 0:500], 0.0
                    )

            def hop3(b):
                """mxo [(wgi k), (j cn)] -> a2 [(j wgi), (k cn)], j-split.
                src = plain [120, 800B] column slice ((wgi,k) merges to a
                contiguous 120-partition dim); dst = 10-partition slice."""
                mxo = mxo_of(b)
                a2_v = et_of(b).rearrange(
                    "(j wgi) (k cn) -> j wgi k cn", wgi=10, cn=2 * P2)
                for j in range(10):
                    # src as a plain [120, 400] rect; element order matches.
                    nc.sync.dma_start(
                        a2_v[j], mxo[:, j * 2 * P2: (j + 1) * 2 * P2])

            def av(b):
                a2 = et_of(b)
                vt = vring[b % 3]
                for jj in range(H // 2):
                    pv = psDV.tile([128, 512], F32, tag="dv")
                    for sub in range(2):
                        k = 2 * jj + sub
                        rows = pv[sub * 64: sub * 64 + 64, 0:P2]
                        tp = (0, sub * 64)
                        for c in range(2):
                            nc.tensor.matmul(
                                rows,
                                vt[0:100, c * 768 + k * 64: c * 768 + (k + 1) * 64],
                                a2[0:100, k * 2 * P2 + c * P2: k * 2 * P2 + c * P2 + P2],
                                start=(c == 0),
                                stop=(c == 1),
                                tile_position=tp,
                            )
                    nc.scalar.copy(
                        aoTall[:, jj * T2 + b * P2: jj * T2 + (b + 1) * P2],
                        pv[:, 0:P2],
                    )

            def proj(g):
                t0 = g * 128
                tsz = min(128, T2 - t0)
                osb = opool.tile([128, 768], BF16, tag="osb")
                for n0, nsz in ((0, 512), (512, 256)):
                    pp = psA.tile([128, 512], F32, tag="a")
                    for kt in range(KT):
                        nc.tensor.matmul(
                            pp[0:tsz, 0:nsz],
                            aoTall[:, kt * T2 + t0: kt * T2 + t0 + tsz],
                            wp[:, kt * 768 + n0: kt * 768 + n0 + nsz],
                            start=(kt == 0),
                            stop=(kt == KT - 1),
                        )
                    nc.vector.tensor_tensor(
                        osb[0:tsz, n0: n0 + nsz],
                        pp[0:tsz, 0:nsz],
                        bp[0:tsz, n0: n0 + nsz],
                        ALU.add,
                    )
                nc.sync.dma_start(out_d[t0: t0 + tsz, :], osb[0:tsz, :])

            # ---- software-pipelined batch loop ----
            vproj(0)
            sdn(0)
            hop2(0)
            vproj(1)
            mix(0)
            hop3(0)
            gdone = 0
            for b in range(BL):
                if b + 1 < BL:
                    sdn(b + 1)
                    hop2(b + 1)
                av(b)
                gr = (P2 * (b + 1)) // 128
                for g in range(gdone, gr):
                    proj(g)
                gdone = gr
                if b + 2 < BL:
                    vproj(b + 2)
                if b + 1 < BL:
                    mix(b + 1)
                    hop3(b + 1)
            for g in range(gdone, (T2 + 127) // 128):
                proj(g)

    nc.compile()
    return nc


def _tile6(a, width):
    """[768, M] -> [128, 6*M] (K-tile-major host layout)."""
    assert a.shape == (768, width)
    return np.ascontiguousarray(
        a.reshape(KT, 128, width).transpose(1, 0, 2).reshape(128, KT * width)
    )


def _to_bf16(a):
    return np.asarray(a, dtype=np.float32).astype(ml_dtypes.bfloat16)


def _posmaps():
    """token m -> padded position p, and p -> m (or -1 for dummies)."""
    pos_of_tok = np.empty(N, np.int64)
    for m in range(N):
        c = 0 if m < 100 else 1
        mm = m - c * 100
        g, ml = mm // 10, mm % 10
        pos_of_tok[m] = c * 100 + ml * 10 + g
    tok_of_pos = np.full(P2, -1, np.int64)
    tok_of_pos[pos_of_tok] = np.arange(N)
    return pos_of_tok, tok_of_pos


_POS_OF_TOK, _TOK_OF_POS = _posmaps()


def _preprocess(inputs):
    x = np.asarray(inputs["x"], np.float32)
    qkv_w = np.asarray(inputs["qkv_w"], np.float32)
    q_bias = np.asarray(inputs["q_bias"], np.float32)
    v_bias = np.asarray(inputs["v_bias"], np.float32)
    sq = np.asarray(inputs["ssf_scale_qkv"], np.float32)
    tq = np.asarray(inputs["ssf_shift_qkv"], np.float32)
    rbt = np.asarray(inputs["rel_bias_table"], np.float32)
    coeff = np.asarray(inputs["bases_coeff"], np.float32)
    proj_w = np.asarray(inputs["proj_w"], np.float32)
    proj_b = np.asarray(inputs["proj_b"], np.float32)
    sp = np.asarray(inputs["ssf_scale_proj"], np.float32)
    tp = np.asarray(inputs["ssf_shift_proj"], np.float32)
    rel_index = np.asarray(inputs["rel_index"], np.int64)

    qkv_bias = np.concatenate([q_bias, np.zeros_like(q_bias), v_bias])
    w_eff = (qkv_w * sq[:, None]).copy()
    b_eff = (qkv_bias * sq + tq).copy()
    w_eff[0:768] *= SCALE
    b_eff[0:768] *= SCALE

    # wqk mt-major: [128, mt*768 + kt*128 + c]
    a = np.ascontiguousarray(w_eff[0:1536].T)          # [768, 1536]
    wqk = np.ascontiguousarray(
        a.reshape(KT, 128, QKM, 128).transpose(1, 2, 0, 3).reshape(128, QKM * 768)
    )
    wvt = _tile6(np.ascontiguousarray(w_eff[1536:].T), 768)
    wp_eff = proj_w * sp[:, None]
    bp_eff = proj_b * sp + tp
    wpt = _tile6(np.ascontiguousarray(wp_eff.T), 768)

    bqk_sb = np.ascontiguousarray(b_eff[0:1536].reshape(QKM, 128).T).astype(np.float32)

    # rel bias in permuted+padded coordinates:
    # relb[p, (h*2+c)*P2 + n] = exp(table[rel_index[qtok(n), ktok(c,p)], h])
    gathered = rbt[rel_index]                      # [query-tok, key-tok, H]
    relb4 = np.zeros((100, H, 2, P2), np.float32)
    q_valid = _TOK_OF_POS >= 0                     # [P2]
    qtok = np.where(q_valid, _TOK_OF_POS, 0)
    for c in range(2):
        ktok_pos = _TOK_OF_POS[c * 100: (c + 1) * 100]   # [100]
        k_valid = ktok_pos >= 0
        ktok = np.where(k_valid, ktok_pos, 0)
        blk = gathered[qtok[None, :], ktok[:, None], :]   # [100, P2, H]
        blk = blk.transpose(0, 2, 1)                      # [100, H, P2]
        blk = np.where(q_valid[None, None, :], blk, 0.0)
        blk = np.where(k_valid[:, None, None], blk, DUMMY_BIAS)
        relb4[:, :, c, :] = blk
    relb = np.exp(relb4.reshape(100, ECOLS))

    # mix = coeff^T * 1.0 + I ; mixblk[wgi*12+h, wgi'*12+k] = d(wgi,wgi')mix[h,k]
    mix = coeff.T + np.eye(H, dtype=np.float32)
    mixblk = np.kron(np.eye(10, dtype=np.float32), mix)
    bv_rep = np.broadcast_to(b_eff[1536:].reshape(1, 768), (128, 768))
    bp_rep = np.broadcast_to(bp_eff.reshape(1, 768), (128, 768))

    common = {
        "wqk": _to_bf16(wqk),
        "wv": _to_bf16(wvt),
        "wp": _to_bf16(wpt),
        "relb": _to_bf16(relb),
        "mixblk": _to_bf16(mixblk),
        "bqk": bqk_sb,
        "bv": _to_bf16(bv_rep),
        "bp": _to_bf16(bp_rep),
    }
    in_maps = []
    for ci in range(NCORES):
        xs = x[ci * BL: (ci + 1) * BL]              # [BL, N, C]
        xp = np.zeros((BL, P2, C), np.float32)
        xp[:, _POS_OF_TOK, :] = xs
        xt = xp.reshape(BL * P2, C).T               # [C, T2]
        # chunk-major xT: per chunk [128, 6*nsz], col = base6 + kt*nsz + n
        parts = []
        for (n0, nsz) in TOKC:
            blk = np.ascontiguousarray(xt[:, n0:n0 + nsz])
            parts.append(blk.reshape(KT, 128, nsz).transpose(1, 0, 2).reshape(128, KT * nsz))
        m = dict(common)
        m["xT"] = _to_bf16(np.concatenate(parts, axis=1))
        in_maps.append(m)
    return in_maps


def _get_compiled():
    if "nc" not in _COMPILED:
        _COMPILED["nc"] = _build_graph()
    return _COMPILED["nc"]


LAST_EXEC_NS = None
LAST_RESULTS = None


def _ensure_ntff_hook():
    """The agent image's antenv package lacks axon_hooks; synthesize it so
    run_bass_kernel_spmd(trace=True) can capture NTFF profiles."""
    import types

    if "antenv.axon_hooks" in sys.modules:
        return
    try:
        sys.path.insert(0, "/root/.axon_site")
        from trn_agent_boot.trn_boot import _ntff_profile_via_ctypes

        hook = _ntff_profile_via_ctypes("/opt/axon/libaxon_pjrt.so")
    except Exception:
        hook = None
    mod = types.ModuleType("antenv.axon_hooks")
    _state = {"hook": hook}
    mod.get_axon_ntff_profile_hook = lambda: _state["hook"]
    mod.set_axon_ntff_profile_hook = lambda h: _state.__setitem__("hook", h)
    sys.modules["antenv.axon_hooks"] = mod


def kernel(**inputs) -> np.ndarray:
    global LAST_EXEC_NS, LAST_RESULTS
    nc = _get_compiled()
    in_maps = _preprocess(inputs)
    from concourse.bass_utils import run_bass_kernel_spmd

    trace = os.environ.get("BASS_KERNEL_PROFILE", "0") == "1"
    if trace:
        _ensure_ntff_hook()
    res = run_bass_kernel_spmd(nc, in_maps, core_ids=list(range(NCORES)), trace=trace)
    LAST_EXEC_NS = res.exec_time_ns
    LAST_RESULTS = res
    outs = []
    for i in range(NCORES):
        o = np.asarray(res.results[i]["out"], dtype=np.float32).reshape(BL, P2, C)
        outs.append(o[:, _POS_OF_TOK, :])           # drop dummies, un-permute
    return np.concatenate(outs, axis=0).astype(np.float32)
